# revision 3
# baseline (speedup 1.0000x reference)
"""3-layer GCN (PyG GCNConv + BatchNorm + ReLU) on 8 Trainium2 NeuronCores.

Strategy (edge-parallel via dst-range sharding):
  - Nodes are sharded into 8 contiguous ranges of 12500 (core i owns dsts
    [12500i, 12500(i+1))); edges (with self-loops appended) are owned by the
    core that owns their dst.  Balanced to ~0.3% by uniform randomness.
  - Per layer, per core:
      hw'      = (h @ W) * dinv[node]          (local GEMM over own shard)
      AllGather-> full fp16 table[100000,128] in every core's HBM
      gather    hw'[src] per edge (batched SWDGE dma_gather, int16 idxs
                relative to one of 4 fixed 25000-row table chunks)
      aggregate per 128-dst window via TensorE one-hot matmul:
                psum[f, dst] += gathered[e, f]^T @ onehot[e, dst]
                where onehot[e, dst_local(e)] = dinv[dst(e)]  (norm folded in;
                the src-side dinv factor is folded into the table)
      y = agg, BN stats partial sums -> AllReduce -> scale/shift
      h_next = relu(y * scale + shift)  (one ACT pass)
  - SPMD: one program for all 8 cores, so the tile schedule (#tiles per
    (chunk, window) cell) is the max over cores; pad slots gather table row 0
    of the chunk and carry all-zero one-hot rows (exactly zero contribution).

Self-loops are plain edges: value dinv[v] in the one-hot against table row
hw[v]*dinv[v] gives exactly the dinv^2[v]*hw[v] self term of the reference.
The bias b is mathematically absorbed by BatchNorm (mean-subtraction), so it
is skipped entirely.
"""

import os
import sys
import time

for _p in ("/opt/trn_rl_repo",):
    if _p not in sys.path:
        sys.path.insert(0, _p)

import numpy as np
from contextlib import ExitStack

import concourse.bacc as bacc
import concourse.bass as bass
import concourse.tile as tile
from concourse import mybir
from concourse.bass_utils import run_bass_kernel_spmd

# problem constants (hardcoded per spec nn_GCNLayers_15607911154176)
N = 100000
D = 128
NCORES = 8
SHARD = 12500           # nodes per core
NWIN = 98               # windows per shard
WIN = 128               # dst window width
SPAD = NWIN * WIN       # 12544, shard padded
NCH = 4                 # src chunks (int16 gather index range)
CHUNK = 25000
# dma_gather tolerates at most 1024 idxs per instruction on HW (larger
# gathers crash the exec unit), so 8 tiles (1024 idxs, 65 ring descs) per call.
GB = int(os.environ.get("KERNEL_GB", "8"))
NQ = int(os.environ.get("KERNEL_NQ", "4"))
GBUFS = int(os.environ.get("KERNEL_GBUFS", "8"))
TBL_F32 = os.environ.get("KERNEL_TBL_F32", "0") == "1"
SINGLE_PACKET = os.environ.get("KERNEL_SP", "1") == "1"
PHASES = os.environ.get("KERNEL_PHASES", "ABC")  # ablation: A, AB, ABC
NLAYERS = int(os.environ.get("KERNEL_NLAYERS", "3"))
BN_EPS = 1e-5
F16 = mybir.dt.float16
F32 = mybir.dt.float32
I16 = mybir.dt.int16


# ---------------------------------------------------------------- schedule

def make_schedule(counts_per_core: np.ndarray):
    """counts_per_core: [NCORES, NCH*NWIN] edge counts per (chunk, window) cell.
    Returns (K, tiles, batches):
      K: [NCH*NWIN] tiles per cell (shared across cores)
      tiles: list of (c, w, k, kmax) in program order
      batches: list of (c, t0, nb) gather batches (tile index ranges)
    """
    K = np.ceil(counts_per_core.max(axis=0) / 128).astype(np.int64)  # [NCH*NWIN]
    tiles = []
    batches = []
    for c in range(NCH):
        chunk_t0 = len(tiles)
        for w in range(NWIN):
            k = int(K[c * NWIN + w])
            for j in range(k):
                tiles.append((c, w, j, k))
        # batches over this chunk's tiles
        t = chunk_t0
        while t < len(tiles):
            nb = min(GB, len(tiles) - t)
            batches.append((c, t, nb))
            t += nb
    return K, tiles, batches


# ---------------------------------------------------------------- device code

def build_program(tiles, batches, ntiles):
    nc = bacc.Bacc("TRN2", target_bir_lowering=False, debug=False,
                   num_devices=NCORES, num_swdge_queues=NQ)

    xT_p = nc.declare_dram_parameter("xT", [128, SPAD], F32, isOutput=False)
    idx_p = nc.declare_dram_parameter("idx", [128, ntiles * 8], I16, isOutput=False)
    oh_p = nc.declare_dram_parameter("oh", [128, ntiles, WIN], F16, isOutput=False)
    dinv_p = nc.declare_dram_parameter("dinv_nm", [128, NWIN], F32, isOutput=False)
    w_ps = [nc.declare_dram_parameter(f"w{l}", [128, 128], F32, isOutput=False)
            for l in range(3)]
    gb_ps = [nc.declare_dram_parameter(f"gb{l}", [128, 2], F32, isOutput=False)
             for l in range(3)]
    out_p = nc.declare_dram_parameter("hT_out", [128, SHARD], F32, isOutput=True)

    TF = F32 if TBL_F32 else F16
    shard_d = [nc.dram_tensor(f"shard{l}", [SPAD, 128], TF) for l in range(3)]
    table_d = [nc.dram_tensor(f"table{l}", [N, 128], TF, addr_space="Shared")
               for l in range(3)]
    stats_in_d = [nc.dram_tensor(f"stats_in{l}", [128, 2], F32) for l in range(3)]
    stats_rd_d = [nc.dram_tensor(f"stats_rd{l}", [128, 2], F32, addr_space="Shared")
                  for l in range(3)]

    rg = [list(range(NCORES))]

    with tile.TileContext(nc) as tc, ExitStack() as ctx:
        persist = ctx.enter_context(tc.tile_pool(name="persist", bufs=1))
        gpool = ctx.enter_context(tc.tile_pool(name="gpool", bufs=GBUFS))
        ohpool = ctx.enter_context(tc.tile_pool(name="ohpool", bufs=4))
        stpool = ctx.enter_context(tc.tile_pool(name="stpool", bufs=3))
        scal = ctx.enter_context(tc.tile_pool(name="scal", bufs=4))
        psum_w = ctx.enter_context(tc.tile_pool(name="psum_w", bufs=4, space="PSUM"))
        psum_g = ctx.enter_context(tc.tile_pool(name="psum_g", bufs=2, space="PSUM"))

        hT = persist.tile([128, SPAD], F32)
        aggT = persist.tile([128, SPAD], F32)
        idx_sb = persist.tile([128, ntiles * 8], I16)
        dinv_sb = persist.tile([128, NWIN], F32)
        w_sb = [persist.tile([128, 128], F32, name=f"wsb{l}", tag=f"w{l}")
                for l in range(3)]
        gb_sb = [persist.tile([128, 2], F32, name=f"gbsb{l}", tag=f"gb{l}")
                 for l in range(3)]
        eps_sb = persist.tile([128, 1], F32)
        bn6 = persist.tile([128, NWIN, 6], F32)

        nc.sync.dma_start(out=hT[:], in_=xT_p[:])
        nc.sync.dma_start(out=idx_sb[:], in_=idx_p[:])
        nc.sync.dma_start(out=dinv_sb[:], in_=dinv_p[:])
        for l in range(3):
            nc.sync.dma_start(out=w_sb[l][:], in_=w_ps[l][:])
            nc.sync.dma_start(out=gb_sb[l][:], in_=gb_ps[l][:])
        nc.vector.memset(eps_sb[:], BN_EPS)

        for l in range(NLAYERS):
            # ---- phase A: table shard = (h @ W) * dinv, node-major fp16
            shard_v = shard_d[l].ap().rearrange("(b p) f -> p b f", p=128)
            for b in range(NWIN):
                ps = psum_g.tile([128, 128], F32)
                nc.tensor.matmul(
                    out=ps[:], lhsT=hT[:, b * 128:(b + 1) * 128], rhs=w_sb[l][:],
                    start=True, stop=True)
                st = stpool.tile([128, 128], TF, tag="stage")
                nc.vector.tensor_scalar_mul(st[:], ps[:], dinv_sb[:, b:b + 1])
                nc.sync.dma_start(out=shard_v[:, b, :], in_=st[:])
            nc.gpsimd.collective_compute(
                "AllGather", mybir.AluOpType.bypass, replica_groups=rg,
                ins=[shard_d[l][:SHARD, :]], outs=[table_d[l][:]])

            # ---- phase B: gather + one-hot aggregate
            if "B" not in PHASES:
                continue
            nc.vector.memset(aggT[:], 0.0)
            bmode = os.environ.get("KERNEL_B_MODE", "full")
            nbatch_lim = int(os.environ.get("KERNEL_NBATCH", "10000"))
            ti = 0  # global tile cursor (tiles are in batch order)
            for bi, (c, t0, nb) in enumerate(batches):
                if bi >= nbatch_lim:
                    break
                g = gpool.tile([128, GB, 128], TF, tag="g")
                if bmode != "mm":
                    nc.gpsimd.dma_gather(
                        g[:, :nb, :],
                        table_d[l][c * CHUNK:(c + 1) * CHUNK, :],
                        idx_sb[:, t0 * 8:(t0 + nb) * 8],
                        nb * 128, nb * 128, 128,
                        queue_num=bi % NQ, single_packet=SINGLE_PACKET,
                    )
                oh = ohpool.tile([128, GB, WIN], F16, tag="oh")
                nc.sync.dma_start(out=oh[:, :nb, :], in_=oh_p[:, t0:t0 + nb, :])
                if bmode == "gather":
                    ti += nb
                    continue
                for t in range(nb):
                    (tc_, tw, tk, tkmax) = tiles[ti]
                    assert tc_ == c
                    if tk == 0:
                        pw = psum_w.tile([128, WIN], F32, tag="pw")
                    nc.tensor.matmul(
                        out=pw[:], lhsT=g[:, t, :], rhs=oh[:, t, :],
                        start=(tk == 0), stop=(tk == tkmax - 1))
                    if tk == tkmax - 1:
                        nc.vector.tensor_add(
                            aggT[:, tw * WIN:(tw + 1) * WIN],
                            aggT[:, tw * WIN:(tw + 1) * WIN],
                            pw[:])
                    ti += 1
            if bmode == "full" and nbatch_lim >= len(batches):
                assert ti == len(tiles)

            # ---- phase C: BN stats + AllReduce + finalize
            if "C" not in PHASES:
                continue
            for sg in range(NWIN):
                nc.vector.bn_stats(out=bn6[:, sg, :], in_=aggT[:, sg * WIN:(sg + 1) * WIN])
            mv = scal.tile([128, 2], F32, tag="mv")
            nc.vector.bn_aggr(out=mv[:], in_=bn6[:])
            # S1 = mean * SPAD ; S2 = (var + mean^2) * SPAD
            st2 = scal.tile([128, 2], F32, tag="st2")
            m2 = scal.tile([128, 1], F32, tag="m2")
            nc.vector.tensor_mul(m2[:], mv[:, 0:1], mv[:, 0:1])
            nc.vector.tensor_scalar_mul(st2[:, 0:1], mv[:, 0:1], float(SPAD))
            nc.vector.tensor_add(m2[:], mv[:, 1:2], m2[:])
            nc.vector.tensor_scalar_mul(st2[:, 1:2], m2[:], float(SPAD))
            nc.sync.dma_start(out=stats_in_d[l][:], in_=st2[:])
            nc.gpsimd.collective_compute(
                "AllReduce", mybir.AluOpType.add, replica_groups=rg,
                ins=[stats_in_d[l][:]], outs=[stats_rd_d[l][:]])
            sr = scal.tile([128, 2], F32, tag="sr")
            nc.sync.dma_start(out=sr[:], in_=stats_rd_d[l][:])

            mu = scal.tile([128, 1], F32, tag="mu")
            var = scal.tile([128, 1], F32, tag="var")
            nc.vector.tensor_scalar_mul(mu[:], sr[:, 0:1], 1.0 / N)
            nc.vector.tensor_scalar_mul(var[:], sr[:, 1:2], 1.0 / N)
            t1 = scal.tile([128, 1], F32, tag="t1")
            nc.vector.tensor_mul(t1[:], mu[:], mu[:])
            nc.vector.tensor_sub(var[:], var[:], t1[:])
            # sd = sqrt(var + eps); r = 1/sd
            sd = scal.tile([128, 1], F32, tag="sd")
            nc.scalar.activation(out=sd[:], in_=var[:],
                                 func=mybir.ActivationFunctionType.Sqrt,
                                 bias=eps_sb[:], scale=1.0)
            r = scal.tile([128, 1], F32, tag="r")
            nc.vector.reciprocal(out=r[:], in_=sd[:])
            scale = scal.tile([128, 1], F32, tag="scale")
            shift = scal.tile([128, 1], F32, tag="shift")
            nc.vector.tensor_mul(scale[:], gb_sb[l][:, 0:1], r[:])
            nc.vector.tensor_mul(t1[:], mu[:], scale[:])
            nc.vector.tensor_sub(shift[:], gb_sb[l][:, 1:2], t1[:])
            # h_next = relu(agg * scale + shift)
            nc.scalar.activation(out=hT[:], in_=aggT[:],
                                 func=mybir.ActivationFunctionType.Relu,
                                 bias=shift[:], scale=scale[:])

        nc.sync.dma_start(out=out_p[:], in_=hT[:, :SHARD])

    nc.compile()
    return nc


# ---------------------------------------------------------------- host side

def preprocess(x, edge_index, dinv):
    """Build per-core input arrays + shared schedule."""
    src = np.asarray(edge_index[0], dtype=np.int64)
    dst = np.asarray(edge_index[1], dtype=np.int64)
    # append self-loops
    loops = np.arange(N, dtype=np.int64)
    src_a = np.concatenate([src, loops])
    dst_a = np.concatenate([dst, loops])

    core = dst_a // SHARD
    dstl = dst_a - core * SHARD
    c = src_a // CHUNK
    srcl = (src_a - c * CHUNK).astype(np.int16)
    w = dstl // WIN
    col = (dstl - w * WIN).astype(np.int64)
    cell = c * NWIN + w

    counts = np.zeros((NCORES, NCH * NWIN), dtype=np.int64)
    per_core = []
    for i in range(NCORES):
        m = core == i
        cell_i = cell[m]
        counts[i] = np.bincount(cell_i, minlength=NCH * NWIN)
        per_core.append((cell_i, srcl[m], col[m], dst_a[m]))

    K, tiles, batches = make_schedule(counts)
    ntiles = len(tiles)
    # slot base per cell, following program tile order (c-major, then w)
    cell_tile_base = np.zeros(NCH * NWIN, dtype=np.int64)
    acc = 0
    for cc in range(NCH):
        for ww in range(NWIN):
            cell_tile_base[cc * NWIN + ww] = acc
            acc += int(K[cc * NWIN + ww])
    assert acc == ntiles

    in_maps = []
    for i in range(NCORES):
        cell_i, srcl_i, col_i, dsta_i = per_core[i]
        order = np.argsort(cell_i, kind="stable")
        cell_s = cell_i[order]
        srcl_s = srcl_i[order]
        col_s = col_i[order]
        dst_s = dsta_i[order]
        # position within cell
        cnts = np.bincount(cell_s, minlength=NCH * NWIN)
        starts = np.zeros(NCH * NWIN, dtype=np.int64)
        starts[1:] = np.cumsum(cnts)[:-1]
        within = np.arange(cell_s.shape[0]) - starts[cell_s]
        slot = cell_tile_base[cell_s] * 128 + within

        idx_arr = np.zeros(ntiles * 128, dtype=np.int16)
        idx_arr[slot] = srcl_s
        oh_arr = np.zeros((128, ntiles, WIN), dtype=np.float16)
        oh_arr[slot % 128, slot // 128, col_s] = dinv[dst_s].astype(np.float16)

        wrapped = idx_arr.reshape(-1, 16).T  # [16, ntiles*8]
        idx_packed = np.tile(wrapped, (8, 1)).copy()  # [128, ntiles*8]
        in_maps.append({"idx": idx_packed, "oh": oh_arr})
    return in_maps, tiles, batches, ntiles


_CACHE = {}
LAST_EXEC_NS = None


def kernel(**inputs) -> np.ndarray:
    x = np.asarray(inputs["x"], dtype=np.float32)
    edge_index = np.asarray(inputs["edge_index"], dtype=np.int64)
    assert x.shape == (N, D) and edge_index.shape[1:] == (1600000,)

    deg = np.bincount(edge_index[1], minlength=N).astype(np.float64) + 1.0
    dinv = (1.0 / np.sqrt(deg)).astype(np.float32)

    in_maps, tiles, batches, ntiles = preprocess(x, edge_index, dinv)

    ck = ("prog", ntiles, tuple(t[0] * 1000 + t[1] for t in tiles[::97]))
    if ck in _CACHE:
        nc = _CACHE[ck]
    else:
        t0 = time.time()
        nc = build_program(tiles, batches, ntiles)
        print(f"[kernel] build+compile {time.time()-t0:.1f}s "
              f"ntiles={ntiles} nbatches={len(batches)}", flush=True)
        _CACHE.clear()
        _CACHE[ck] = nc

    # dinv node-major [128, NWIN] per core: dinv_nm[p, b] = dinv[i*SHARD + b*128 + p]
    dinv_pad = np.zeros(NCORES * SPAD, dtype=np.float32)
    for i in range(NCORES):
        dinv_pad[i * SPAD:i * SPAD + SHARD] = dinv[i * SHARD:(i + 1) * SHARD]
    for i in range(NCORES):
        im = in_maps[i]
        xT = np.zeros((128, SPAD), dtype=np.float32)
        xT[:, :SHARD] = x[i * SHARD:(i + 1) * SHARD].T
        im["xT"] = xT
        im["dinv_nm"] = dinv_pad[i * SPAD:(i + 1) * SPAD].reshape(NWIN, 128).T.copy()
        for l in range(3):
            im[f"w{l}"] = np.asarray(inputs[f"W{l}"], dtype=np.float32)
            gamma = np.asarray(inputs[f"gamma{l}"], dtype=np.float32)
            beta = np.asarray(inputs[f"beta{l}"], dtype=np.float32)
            im[f"gb{l}"] = np.stack([gamma, beta], axis=1).copy()

    t0 = time.time()
    trace = os.environ.get("KERNEL_TRACE", "0") == "1"
    tkw = {}
    if trace:
        tdir = os.environ.get("KERNEL_TRACE_DIR", "/tmp/ktrace")
        os.makedirs(tdir, exist_ok=True)
        tkw = dict(trace=True, tmpdir=tdir)
    res = run_bass_kernel_spmd(nc, in_maps, list(range(NCORES)), **tkw)
    print(f"[kernel] run {time.time()-t0:.1f}s", flush=True)
    global LAST_EXEC_NS
    LAST_EXEC_NS = res.exec_time_ns
    if LAST_EXEC_NS is not None:
        print(f"HW exec time: {LAST_EXEC_NS} ns", flush=True)

    out = np.empty((N, D), dtype=np.float32)
    for i in range(NCORES):
        out[i * SHARD:(i + 1) * SHARD] = res.results[i]["hT_out"].T
    return out


if __name__ == "__main__":
    # quick self-drive with random inputs (not the reference check)
    rng = np.random.default_rng(0)
    ins = {
        "x": rng.standard_normal((N, D)).astype(np.float32),
        "edge_index": rng.integers(0, N, size=(2, 1600000)),
    }
    for l in range(3):
        ins[f"W{l}"] = (rng.random((128, 128), dtype=np.float32) - 0.5) / np.sqrt(128)
        ins[f"b{l}"] = np.zeros(128, np.float32)
        ins[f"gamma{l}"] = np.ones(128, np.float32)
        ins[f"beta{l}"] = np.zeros(128, np.float32)
    out = kernel(**ins)
    print("out", out.shape, out.dtype, float(np.abs(out).max()))

